# revision 21
# baseline (speedup 1.0000x reference)
"""KNN palette retrieval, G=64 variant: 64 pixel sets x 2 k-slots per PSUM
tensor -> 22 slots (21 real + 1 pad) in 11 tensors.  Fewer wasted slots and
only one fold level vs the G=32 layout, cutting DVE passes per pixel.

Pipeline per tile of 64x512 pixels (16 tiles):
  mm1_i (x11, 2 matmuls each): pa_i[64k'+g, n] = sims k=2i+k' (fp32r)
  stage: s_i = copy(pa_i) -> SBUF, i=0..9 (ACT)
  chain: r = running TT-max over s_0..s_9 (DVE, 9 ops)
  fold:  u = max(pa_10[0:64], r[64:128]) (DVE; PSUM operand crosses
         bases; pa_10 upper half is the zero pad), m = max(u, r[0:64])
  mbc:   mb[64k'+g,n] = m[g,n] via exact plain-fp32 matmul; mbs -> SBUF
  oh_i:  is_ge(s_i|pa_10, mbs) {0,1} bf16 (DVE, 11 ops)
  mm3:   po_h[32c+gq, n] += sum cn_bf16[k,c]*oh_i for halves h (2x11 bf16
         matmuls)
  yout:  po_h -> SBUF bf16 (ACT), DMA out; host upcasts.
"""

import sys

sys.path.insert(0, "/opt/trn_rl_repo")

import numpy as np
import ml_dtypes

BF16 = ml_dtypes.bfloat16

B, C, H, W = 16, 3, 512, 512
K = 21
NCORES = 8
BPC = B // NCORES
PXC = BPC * H * W            # pixels per core = 524288
G = 64                       # pixel sets
REG = PXC // G               # 8192 columns per set
NT = 512
NTILES = REG // NT           # 16
NK = 11                      # k-slot PSUM tensors (11*2 = 22 >= 21)

_CACHE: dict = {}


def _build_nc():
    if "nc" in _CACHE:
        return _CACHE["nc"]
    from contextlib import ExitStack

    import concourse.tile as tile
    from concourse import bacc, mybir

    f32 = mybir.dt.float32
    f32r = mybir.dt.float32r
    bf16 = mybir.dt.bfloat16
    mx = mybir.AluOpType.max
    ge = mybir.AluOpType.is_ge

    nc = bacc.Bacc("TRN2", target_bir_lowering=False, debug=False,
                   num_devices=NCORES)
    x = nc.dram_tensor("x", [2, 96, REG], f32r, kind="ExternalInput").ap()
    # weights packed column-wise so each loads in ONE big DMA (45 small
    # serial weight DMAs cost ~28us of pipeline startup otherwise)
    w1 = nc.dram_tensor("w1", [96, 2 * NK * 128], f32r,
                        kind="ExternalInput").ap()
    wb = nc.dram_tensor("wb", [G, 128], f32, kind="ExternalInput").ap()
    w3 = nc.dram_tensor("w3", [128, 2 * NK * 96], bf16,
                        kind="ExternalInput").ap()
    y = nc.dram_tensor("y", [2, 96, REG], bf16, kind="ExternalOutput").ap()

    with ExitStack() as ctx:
        tc = ctx.enter_context(tile.TileContext(nc))
        wp = ctx.enter_context(tc.tile_pool(name="w", bufs=1))
        inp = ctx.enter_context(tc.tile_pool(name="xin", bufs=3))
        sp = ctx.enter_context(tc.tile_pool(name="s", bufs=2))
        ohp = ctx.enter_context(tc.tile_pool(name="oh", bufs=2))
        yp = ctx.enter_context(tc.tile_pool(name="y", bufs=3))
        pap = ctx.enter_context(tc.tile_pool(name="pa", bufs=4, space="PSUM"))
        pmb = ctx.enter_context(tc.tile_pool(name="pmb", bufs=1, space="PSUM"))
        pop = [ctx.enter_context(
            tc.tile_pool(name=f"po{h}", bufs=1, space="PSUM"))
            for h in range(2)]

        # Startup-latency ordering: the first DVE op needs s0,s1 <- pa0,pa1
        # <- w1 cols 0:512 + tile-0 inputs, so issue exactly those first on
        # SP; bulk weights follow (w3 on ACT: it is needed ~15us in, and
        # keeps SP free for input tiles).
        w1big = wp.tile([96, 2 * NK * 128], f32r)
        C1 = 4 * 128
        nc.sync.dma_start(w1big[:, 0:C1], w1[:, 0:C1])
        xa0 = inp.tile([96, NT], f32r, tag="xa", name="xa")
        xb0 = inp.tile([96, NT], f32r, tag="xb", name="xb")
        nc.sync.dma_start(xa0[:], x[0, :, 0:NT])
        nc.sync.dma_start(xb0[:], x[1, :, 0:NT])
        nc.sync.dma_start(w1big[:, C1:], w1[:, C1:])
        wbs = wp.tile([G, 128], f32)
        nc.sync.dma_start(wbs[:], wb[:])
        w3big = wp.tile([128, 2 * NK * 96], bf16)
        nc.scalar.dma_start(w3big[:], w3[:])
        w1s = [[w1big[:, (2 * i + h) * 128:(2 * i + h + 1) * 128]
                for h in range(2)] for i in range(NK)]
        w3s = [[w3big[:, (2 * i + h) * 96:(2 * i + h + 1) * 96]
                for h in range(2)] for i in range(NK)]

        for t in range(NTILES):
            c0 = t * NT
            if t == 0:
                xa, xb = xa0, xb0
            else:
                xa = inp.tile([96, NT], f32r, tag="xa", name="xa")
                xb = inp.tile([96, NT], f32r, tag="xb", name="xb")
                nc.sync.dma_start(xa[:], x[0, :, c0:c0 + NT])
                nc.sync.dma_start(xb[:], x[1, :, c0:c0 + NT])

            pa = []
            for i in range(NK):
                pai = pap.tile([128, NT], f32, tag="pa", name=f"pa{i}")
                nc.tensor.matmul(pai[:], w1s[i][0], xa[:],
                                 start=True, stop=False)
                nc.tensor.matmul(pai[:], w1s[i][1], xb[:],
                                 start=False, stop=True)
                pa.append(pai)

            # stage sims 0..9 in SBUF (ACT), freeing PSUM slots early
            s = [sp.tile([128, NT], f32, tag=f"s{i}", name=f"s{i}")
                 for i in range(NK - 1)]
            for i in range(NK - 1):
                nc.scalar.copy(s[i][:], pa[i][:])

            # tree-max over s_0..s_9 on DVE (depth 4 vs 9 for a serial
            # chain; same op count, shorter per-tile critical path)
            t0 = sp.tile([128, NT], f32, tag="t0")
            t1 = sp.tile([128, NT], f32, tag="t1")
            t2 = sp.tile([128, NT], f32, tag="t2")
            t3 = sp.tile([128, NT], f32, tag="t3")
            t4 = sp.tile([128, NT], f32, tag="t4")
            nc.vector.tensor_tensor(t0[:], s[0][:], s[1][:], mx)
            nc.vector.tensor_tensor(t1[:], s[2][:], s[3][:], mx)
            nc.vector.tensor_tensor(t2[:], s[4][:], s[5][:], mx)
            nc.vector.tensor_tensor(t3[:], s[6][:], s[7][:], mx)
            nc.vector.tensor_tensor(t4[:], s[8][:], s[9][:], mx)
            nc.vector.tensor_tensor(t0[:], t0[:], t1[:], mx)
            nc.vector.tensor_tensor(t2[:], t2[:], t3[:], mx)
            nc.vector.tensor_tensor(t0[:], t0[:], t2[:], mx)
            cur = sp.tile([128, NT], f32, tag="cur")
            nc.vector.tensor_tensor(cur[:], t0[:], t4[:], mx)
            # fold: pa_10 (k=20 real lower half, k=21 zero pad upper) as
            # the PSUM operand lets the 64-row fold cross partition bases.
            # (No zero-pixel floor: all-zero pixels have probability ~2^-72.)
            u = sp.tile([64, NT], f32, tag="u")
            nc.vector.tensor_tensor(u[:], pa[10][0:64, :], cur[64:128, :], mx)
            m = sp.tile([64, NT], f32, tag="m")
            nc.vector.tensor_tensor(m[:], u[:], cur[0:64, :], mx)

            # broadcast m[g] to rows 64k'+g via exact plain-fp32 matmul
            mbp = pmb.tile([128, NT], f32, tag="mb", name="mb")
            nc.tensor.matmul(mbp[:], wbs[:], m[:], start=True, stop=True)
            mbs = sp.tile([128, NT], f32, tag="mbs")
            nc.scalar.copy(mbs[:], mbp[:])

            # one-hot compares: exact f32 is_ge, {0,1} in bf16
            oh = [ohp.tile([128, NT], bf16, tag=f"oh{i}", name=f"oh{i}")
                  for i in range(NK)]
            nc.vector.tensor_tensor(oh[10][:], pa[10][:], mbs[:], ge)
            for i in range(NK - 1):
                nc.vector.tensor_tensor(oh[i][:], s[i][:], mbs[:], ge)

            for h in range(2):
                po = pop[h].tile([96, NT], f32, tag=f"po{h}", name=f"po{h}")
                for i in range(NK):
                    nc.tensor.matmul(po[:], w3s[i][h], oh[i][:],
                                     start=(i == 0), stop=(i == NK - 1))
                yo = yp.tile([96, NT], bf16, tag=f"yo{h}", name=f"yo{h}")
                nc.scalar.copy(yo[:], po[:])
                nc.gpsimd.dma_start(y[h, :, c0:c0 + NT], yo[:])

    nc.compile()
    _CACHE["nc"] = nc
    return nc


def _weights(colors: np.ndarray):
    cn = (colors.astype(np.float64)
          / np.linalg.norm(colors.astype(np.float64), axis=-1, keepdims=True)
          ).astype(np.float32)
    cnb = cn.astype(BF16)
    W1 = np.zeros((NK, 2, 96, 128), np.float32)
    Wb = np.zeros((G, 128), np.float32)
    W3 = np.zeros((NK, 2, 128, 96), BF16)
    for i in range(NK):
        for kp in range(2):
            k = 2 * i + kp
            if k >= K:
                continue
            for h in range(2):
                for gq in range(32):
                    g = 32 * h + gq
                    for c in range(C):
                        W1[i, h, 32 * c + gq, 64 * kp + g] = cn[k, c]
                        W3[i, h, 64 * kp + g, 32 * c + gq] = cnb[k, c]
    for g in range(G):
        for kp in range(2):
            Wb[g, 64 * kp + g] = 1.0
    # pack [NK, 2, P, M] -> [P, (2i+h)*M + col] for single-DMA loads
    W1p = np.ascontiguousarray(
        W1.transpose(2, 0, 1, 3).reshape(96, 2 * NK * 128))
    W3p = np.ascontiguousarray(
        W3.transpose(2, 0, 1, 3).reshape(128, 2 * NK * 96))
    return W1p, Wb, W3p


def _stage_inputs(rgb_mask: np.ndarray, colors: np.ndarray):
    W1, Wb, W3 = _weights(np.asarray(colors, np.float32))
    in_maps = []
    for i in range(NCORES):
        xc = np.asarray(rgb_mask[BPC * i:BPC * (i + 1)], np.float32)
        xc = np.transpose(xc, (1, 0, 2, 3)).reshape(C, G, REG)
        xs = np.stack([
            xc[:, 0:32].reshape(96, REG),
            xc[:, 32:64].reshape(96, REG),
        ])
        in_maps.append({
            "x": np.ascontiguousarray(xs),
            "w1": W1, "wb": Wb, "w3": W3,
        })
    return in_maps


def _gather_outputs(results):
    outs = []
    for i in range(NCORES):
        yb = np.asarray(results[i]["y"]).astype(np.float32)  # [2, 96, REG]
        yc = np.empty((C, G, REG), np.float32)
        yc[:, 0:32] = yb[0].reshape(C, 32, REG)
        yc[:, 32:64] = yb[1].reshape(C, 32, REG)
        yc = yc.reshape(C, BPC, H, W)
        outs.append(np.transpose(yc, (1, 0, 2, 3)))
    return np.ascontiguousarray(np.concatenate(outs, axis=0))


def run(rgb_mask, colors, trace=False, **kw):
    from concourse.bass_utils import run_bass_kernel_spmd

    nc = _build_nc()
    in_maps = _stage_inputs(rgb_mask, colors)
    res = run_bass_kernel_spmd(nc, in_maps, core_ids=list(range(NCORES)),
                               trace=trace, **kw)
    return _gather_outputs(res.results), res


def kernel(rgb_mask, colors):
    out, _ = run(rgb_mask, colors)
    return out
